# revision 13
# baseline (speedup 1.0000x reference)
"""Causal self-attention (B=4, T=1024, D=2048, H=16) on 8 trn2 NeuronCores.

Sharding: data-parallel over batch (4) x tensor-parallel over heads (2).
Core c handles batch b = c//2, head-half hh = c%2 (heads hh*8 .. hh*8+8).

All matmuls bf16 (inputs cast host-side), fp32 PSUM accumulation:
  v      [t, c]  : lhsT = xT tile [k,t], rhs = wv [k,c]   (first, all heads)
  then per head h (pipelined):
    qT/kT [d, t] : lhsT = w_{q,k} tile [k,c=h], rhs = xT [k,t]
    sT    [tk,tq]: lhsT = kT block, rhs = qT slice (causal: tq >= 128*j only)
    pT    = exp(scale * sT) via ACT (no max-subtraction; |scaled scores| ~ 6)
    diag blocks masked multiplicatively with an upper-triangular 0/1 mask
    yT    [d, tq] += v_j-gemm: lhsT = v block, rhs = pT block (PSUM accum)
    r     [1, tq] += ones^T @ pT (softmax row sums, same rhs stream)
    yT_norm = yT * bcast(1/r) (DVE copy -> GpSimd bcast -> DVE approx-recip)
    pairwise AllGather of this head's yT (overlaps later heads' compute)
  out    [t, c_half]: proj in two column halves; first half kk-major
    (tolerates late peer AllGather), second half m-major (drains output
    DMAs progressively so the kernel has no un-overlapped tail).
    Own heads' yT feed the proj straight from local DRAM (no AllGather
    dependency); wp rows are pre-swapped host-side into [own|peer] order
    so the SPMD program is parity-free, and peer tiles are fetched with
    parity-conditional DMAs.
Host side: slice/transpose/cast inputs per core, concat outputs.
"""

import numpy as np
import ml_dtypes

import concourse.bass as bass
import concourse.mybir as mybir
import concourse.tile as tile
from concourse import bacc
from concourse.bass_utils import run_bass_kernel_spmd
from concourse.dve_ops import RECIPROCAL_APPROX_NR

B, T, D = 4, 1024, 2048
H, DH = 16, 128
N_CORES = 8
TP = 2                      # head-halves per batch
HPC = H // TP               # heads per core = 8
CPC = HPC * DH              # channels per core = 1024
KC = D // 128               # contraction chunks = 16
XCH = 4                     # x/wv DMA chunks (4 k-slices each)
SCALE = 1.0 / float(np.sqrt(DH))

F32 = mybir.dt.float32
BF16 = mybir.dt.bfloat16

PAIRS = [[2 * i, 2 * i + 1] for i in range(B)]


def build_kernel():
    nc = bacc.Bacc("TRN2", target_bir_lowering=False, debug=False,
                   num_devices=N_CORES)

    xT_ap = nc.dram_tensor("xT", [D, T], BF16, kind="ExternalInput").ap()
    wq_ap = nc.dram_tensor("wq", [D, CPC], BF16, kind="ExternalInput").ap()
    wk_ap = nc.dram_tensor("wk", [D, CPC], BF16, kind="ExternalInput").ap()
    wv_ap = nc.dram_tensor("wv", [D, CPC], BF16, kind="ExternalInput").ap()
    wp_ap = nc.dram_tensor("wp", [D, CPC], BF16, kind="ExternalInput").ap()
    maskT_ap = nc.dram_tensor("maskT", [128, 128], BF16,
                              kind="ExternalInput").ap()
    out_ap = nc.dram_tensor("out", [T, CPC], F32, kind="ExternalOutput").ap()

    with tile.TileContext(nc) as tc:
        _body(nc, tc, xT_ap, wq_ap, wk_ap, wv_ap, wp_ap, maskT_ap, out_ap)
    nc.compile()
    return nc


def _body(nc, tc, xT_ap, wq_ap, wk_ap, wv_ap, wp_ap, maskT_ap, out_ap):
    Exp = mybir.ActivationFunctionType.Exp
    mult = mybir.AluOpType.mult

    with tc.tile_pool(name="const", bufs=1) as const, \
         tc.tile_pool(name="dram", bufs=HPC, space="DRAM") as dram, \
         tc.tile_pool(name="xa", bufs=8) as xa, \
         tc.tile_pool(name="wvs", bufs=4) as wvsp, \
         tc.tile_pool(name="wqk", bufs=4) as wqkp, \
         tc.tile_pool(name="wp", bufs=1) as wpp, \
         tc.tile_pool(name="vv", bufs=8) as vvp, \
         tc.tile_pool(name="qk", bufs=4) as qkp, \
         tc.tile_pool(name="pt", bufs=10) as ptp, \
         tc.tile_pool(name="yt", bufs=2) as ytp, \
         tc.tile_pool(name="nrm", bufs=1) as nrm, \
         tc.tile_pool(name="yf", bufs=2 * HPC) as yfp, \
         tc.tile_pool(name="osb", bufs=2) as osb, \
         tc.tile_pool(name="ps", bufs=8, space="PSUM") as pp:

        _psn = [0]

        def psum():
            _psn[0] += 1
            return pp.tile([128, 512], F32, tag="ps", name=f"ps{_psn[0]}")

        maskT = const.tile([128, 128], BF16, tag="maskT", name="maskT")
        ones_f32 = const.tile([128, 1], F32, tag="ones_f32", name="ones_f32")
        nc.vector.memset(ones_f32, 1.0)
        ones_col = const.tile([128, 1], BF16, tag="ones_col", name="ones_col")
        nc.scalar.copy(out=ones_col, in_=ones_f32)

        # ---- input DMAs ----
        # Per-queue DMA bandwidth is the startup bottleneck: spread the
        # x / wv chunks round-robin over the three DMA-capable queues
        # (sync/scalar/gpsimd) ordered by first-use time.
        xts = [xa.tile([128, 2, T], BF16, tag="xT", name=f"xt{c}")
               for c in range(8)]

        def load_x(eng, c):
            eng.dma_start(
                out=xts[c],
                in_=xT_ap[256 * c:256 * (c + 1), :].rearrange(
                    "(k p) t -> p k t", p=128))

        def xk(k):
            return xts[k // 2][:, k % 2, :]


        def load_wqk(hp):     # loads q+k weights for heads 2*hp, 2*hp+1
            tiles = []
            for w_ap, nm in ((wq_ap, "q"), (wk_ap, "k")):
                wt = wqkp.tile([128, KC, 256], BF16, tag="wqk",
                               name=f"w{nm}p{hp}")
                nc.gpsimd.dma_start(
                    out=wt,
                    in_=w_ap[:, 256 * hp:256 * (hp + 1)].rearrange(
                        "(k p) j -> p k j", p=128))
                tiles.append(wt)
            return tiles

        yt_loc = [dram.tile([128, T], BF16, tag="ytl", name=f"ytl{h}")
                  for h in range(HPC - 2)]
        yt_all = [dram.tile([TP, 128, T], BF16, tag="yta", name=f"yta{h}")
                  for h in range(HPC - 2)]
        yt67_loc = dram.tile([2, 128, T], BF16, tag="ytl67", name="ytl67")
        yt67_all = dram.tile([TP, 2, 128, T], BF16, tag="yta67",
                             name="yta67")

        vv = [vvp.tile([128, CPC], BF16, tag="vv", name=f"vv{j}")
              for j in range(8)]

        # ---- v natural [t, c]: stationary xT slices, moving wv ----
        # wv streamed in half-column chunks, re-fetched per ch pass
        wvts = [wvsp.tile([128, KC // XCH, 512], BF16, tag="wv",
                          name=f"wv{ch}_{c}")
                for ch in range(2) for c in range(XCH)]

        def load_wv(eng, i):
            ch, c = i // XCH, i % XCH
            eng.dma_start(
                out=wvts[i],
                in_=wv_ap[512 * c:512 * (c + 1),
                          512 * ch:512 * (ch + 1)].rearrange(
                    "(k p) j -> p k j", p=128))

        # interleaved by need time: x chunk c feeds k=2c..2c+1 (~3.4us per
        # chunk), wv chunk i feeds 4 k-iterations of one ch pass
        load_x(nc.sync, 0)
        load_wv(nc.scalar, 0)
        load_x(nc.gpsimd, 1)
        load_wv(nc.sync, 1)
        load_x(nc.scalar, 2)
        load_x(nc.gpsimd, 3)
        load_x(nc.sync, 4)
        load_wv(nc.scalar, 2)
        load_x(nc.gpsimd, 5)
        load_x(nc.sync, 7)
        load_x(nc.scalar, 6)
        load_wv(nc.gpsimd, 3)
        load_wv(nc.sync, 4)
        load_wv(nc.scalar, 5)
        load_wv(nc.gpsimd, 6)
        load_wv(nc.sync, 7)
        nc.gpsimd.dma_start(out=maskT, in_=maskT_ap)

        for ch in range(2):
            ps = [psum() for _ in range(8)]
            for k in range(KC):
                rhs = wvts[4 * ch + k // 4][:, k % 4, :]
                for tch in range(8):
                    nc.tensor.matmul(
                        ps[tch], xk(k)[:, 128 * tch:128 * (tch + 1)], rhs,
                        start=(k == 0), stop=(k == KC - 1))
            for tch in range(8):
                nc.scalar.copy(
                    out=vv[tch][:, 512 * ch:512 * (ch + 1)], in_=ps[tch])

        # ---- per-head: qk gemm + attention + per-head AllGather ----
        pid = nc.sync.partition_id()
        is_odd = pid % 2
        is_even = 1 - is_odd
        yfull = [None] * (2 * HPC)      # [h] = own head h, [HPC+h] = peer
        wts_next = load_wqk(0)
        # wp is not needed until the projection: issue it on the scalar
        # queue, which only reaches this instruction after the v-proj
        # copies -> it cannot steal startup DMA bandwidth
        wpt = wpp.tile([128, KC, CPC], BF16, tag="wp", name="wp")
        nc.scalar.dma_start(
            out=wpt, in_=wp_ap.rearrange("(k p) j -> p k j", p=128))
        for h in range(HPC):
            if h % 2 == 0:
                wts_cur = wts_next
                if h + 2 < HPC:
                    wts_next = load_wqk(h // 2 + 1)
            ho = 128 * (h % 2)
            qkT = []
            for wt, nm in ((wts_cur[0], "q"), (wts_cur[1], "k")):
                outT = qkp.tile([128, T], BF16, tag="qkT", name=f"{nm}T{h}")
                qkT.append(outT)
                for th in range(2):
                    ps = psum()
                    for k in range(KC):
                        nc.tensor.matmul(
                            ps, wt[:, k, ho:ho + 128],
                            xk(k)[:, 512 * th:512 * (th + 1)],
                            start=(k == 0), stop=(k == KC - 1))
                    nc.scalar.copy(
                        out=outT[:, 512 * th:512 * (th + 1)], in_=ps)
            qTh, kTh = qkT

            pts = []
            for j in range(8):
                pt = ptp.tile([128, T], BF16, tag="pT", name=f"pT{h}_{j}")
                pts.append(pt)
                if j < 4:
                    pieces = [(128 * j, 512 - 128 * j), (512, 512)]
                else:
                    pieces = [(128 * j, 1024 - 128 * j)]
                for off, cw in pieces:
                    sp = psum()
                    nc.tensor.matmul(
                        sp[:, :cw], kTh[:, 128 * j:128 * (j + 1)],
                        qTh[:, off:off + cw], start=True, stop=True)
                    nc.scalar.activation(
                        out=pt[:, off - 128 * j:off - 128 * j + cw],
                        in_=sp[:, :cw], func=Exp, scale=SCALE)
                # causal mask on the diagonal block (local cols 0:128)
                nc.vector.tensor_tensor(
                    out=pt[:, 0:128], in0=pt[:, 0:128], in1=maskT, op=mult)

            yt = ytp.tile([128, T], BF16, tag="yT", name=f"yt{h}")
            for g in range(2):
                tq0 = 512 * g
                jmax = 4 * (g + 1)
                yp = psum()
                rp = psum()
                for j in range(jmax):
                    lo = max(tq0, 128 * j)          # first valid tq
                    w = tq0 + 512 - lo
                    rhs = pts[j][:, lo - 128 * j:lo - 128 * j + w]
                    vblk = vv[j][:, 128 * h:128 * (h + 1)]
                    nc.tensor.matmul(
                        yp[:, lo - tq0:lo - tq0 + w], vblk, rhs,
                        start=(j == 0), stop=(j == jmax - 1))
                    nc.tensor.matmul(
                        rp[0:1, lo - tq0:lo - tq0 + w], ones_col, rhs,
                        start=(j == 0), stop=(j == jmax - 1))
                # softmax denom: psum -> sbuf -> bcast -> recip -> mult
                r_sb = nrm.tile([1, 512], F32, tag="r_sb", name=f"r_sb{h}_{g}")
                nc.vector.tensor_copy(out=r_sb, in_=rp[0:1, :])
                r_bc = nrm.tile([128, 512], F32, tag="r_bc", name=f"r_bc{h}_{g}")
                nc.gpsimd.partition_broadcast(r_bc, r_sb)
                rec = nrm.tile([128, 512], F32, tag="rec", name=f"rec{h}_{g}")
                nc.vector.reciprocal_approx_fast(out=rec, in_=r_bc)
                nc.vector._custom_dve(
                    RECIPROCAL_APPROX_NR, out=rec, in0=r_bc, in1=rec, s0=2.0)
                nc.vector.tensor_tensor(out=yt[:, tq0:tq0 + 512],
                                        in0=yp, in1=rec, op=mult)
            # ship this head's yT to the pair as soon as it's done; own
            # copy feeds the proj from local DRAM without any AG wait
            def fetch_peer(h2, src_aps):
                t2 = yfp.tile([128, T], BF16, tag="yfull",
                              name=f"yfpeer{h2}")
                nc.sync.dma_start(out=t2, in_=src_aps[1], cond=is_even)
                nc.sync.dma_start(out=t2, in_=src_aps[0], cond=is_odd)
                yfull[HPC + h2] = t2

            def fetch_own(h2, src_ap):
                t2 = yfp.tile([128, T], BF16, tag="yfull",
                              name=f"yfown{h2}")
                nc.sync.dma_start(out=t2, in_=src_ap)
                yfull[h2] = t2

            if h < HPC - 2:
                nc.sync.dma_start(out=yt_loc[h], in_=yt)
                fetch_own(h, yt_loc[h])
                nc.gpsimd.collective_compute(
                    "AllGather", mybir.AluOpType.bypass,
                    replica_groups=PAIRS,
                    ins=[yt_loc[h].opt()], outs=[yt_all[h].opt()])
                fetch_peer(h, [yt_all[h][0], yt_all[h][1]])
            else:
                nc.sync.dma_start(out=yt67_loc[h - (HPC - 2)], in_=yt)
                fetch_own(h, yt67_loc[h - (HPC - 2)])
                if h == HPC - 1:
                    nc.gpsimd.collective_compute(
                        "AllGather", mybir.AluOpType.bypass,
                        replica_groups=PAIRS,
                        ins=[yt67_loc.opt()], outs=[yt67_all.opt()])
                    for hh in range(2):
                        fetch_peer(HPC - 2 + hh,
                                   [yt67_all[0][hh], yt67_all[1][hh]])

        # ---- output projection out[t, c_half] = yT_full @ wp cols ----
        # consume own heads first (available without AG), peers after, in
        # head order (matches AG completion); wp rows are [own|peer].
        # first column half: kk-major (tolerates the last AllGather landing
        # mid-phase; psum holds all 8 t-chunks)
        ps0 = [psum() for _ in range(8)]
        for idx in range(2 * HPC):
            rhs = wpt[:, idx, 0:512]
            for m in range(8):
                nc.tensor.matmul(
                    ps0[m], yfull[idx][:, 128 * m:128 * (m + 1)], rhs,
                    start=(idx == 0), stop=(idx == 2 * HPC - 1))
        for m in range(8):
            ot = osb.tile([128, 512], F32, tag="ot", name=f"ot0_{m}")
            nc.scalar.copy(out=ot, in_=ps0[m])
            nc.sync.dma_start(
                out=out_ap[128 * m:128 * (m + 1), 0:512], in_=ot)
        # second column half: m-major (each t-chunk drains + DMAs as soon
        # as its 16 accumulations finish -> no output tail)
        for m in range(8):
            ps = psum()
            for idx in range(2 * HPC):
                nc.tensor.matmul(
                    ps, yfull[idx][:, 128 * m:128 * (m + 1)],
                    wpt[:, idx, 512:1024],
                    start=(idx == 0), stop=(idx == 2 * HPC - 1))
            ot = osb.tile([128, 512], F32, tag="ot", name=f"ot1_{m}")
            nc.scalar.copy(out=ot, in_=ps)
            nc.sync.dma_start(
                out=out_ap[128 * m:128 * (m + 1), 512:1024], in_=ot)


_NC_CACHE = None


def _get_nc():
    global _NC_CACHE
    if _NC_CACHE is None:
        _NC_CACHE = build_kernel()
    return _NC_CACHE


def kernel(x, w_qkv, w_proj, _trace=False, _trace_kwargs=None):
    bf16 = ml_dtypes.bfloat16
    x = np.asarray(x, dtype=np.float32)
    w_qkv = np.asarray(w_qkv, dtype=np.float32)
    w_proj = np.asarray(w_proj, dtype=np.float32)

    maskT = np.triu(np.ones((128, 128), dtype=np.float32)).astype(bf16)

    in_maps = []
    for c in range(N_CORES):
        b, hh = c // TP, c % TP
        cols = slice(hh * CPC, (hh + 1) * CPC)
        in_maps.append({
            "xT": np.ascontiguousarray(x[b].T).astype(bf16),
            "wq": np.ascontiguousarray(w_qkv[:, :D][:, cols]).astype(bf16),
            "wk": np.ascontiguousarray(
                w_qkv[:, D:2 * D][:, cols]).astype(bf16),
            "wv": np.ascontiguousarray(w_qkv[:, 2 * D:][:, cols]).astype(bf16),
            "wp": np.ascontiguousarray(np.concatenate(
                [w_proj[hh * CPC:(hh + 1) * CPC, cols],
                 w_proj[(1 - hh) * CPC:(2 - hh) * CPC, cols]],
                axis=0)).astype(bf16),
            "maskT": maskT,
        })

    nc = _get_nc()
    res = run_bass_kernel_spmd(nc, in_maps, list(range(N_CORES)),
                               trace=_trace, **(_trace_kwargs or {}))

    out = np.empty((B, T, D), dtype=np.float32)
    for c in range(N_CORES):
        b, hh = c // TP, c % TP
        out[b, :, hh * CPC:(hh + 1) * CPC] = res.results[c]["out"]
    if _trace:
        return out, res
    return out


# revision 14
# speedup vs baseline: 1.0359x; 1.0359x over previous
"""Causal self-attention (B=4, T=1024, D=2048, H=16) on 8 trn2 NeuronCores.

Sharding: data-parallel over batch (4) x tensor-parallel over heads (2).
Core c handles batch b = c//2, head-half hh = c%2 (heads hh*8 .. hh*8+8).

All matmuls bf16 (inputs cast host-side), fp32 PSUM accumulation:
  v      [t, c]  : lhsT = xT tile [k,t], rhs = wv [k,c]   (first, all heads)
  then per head h (pipelined):
    qT/kT [d, t] : lhsT = w_{q,k} tile [k,c=h], rhs = xT [k,t]
    sT    [tk,tq]: lhsT = kT block, rhs = qT slice (causal: tq >= 128*j only)
    pT    = exp(scale * sT) via ACT (no max-subtraction; |scaled scores| ~ 6)
    diag blocks masked multiplicatively with an upper-triangular 0/1 mask
    yT    [d, tq] += v_j-gemm: lhsT = v block, rhs = pT block (PSUM accum)
    r     [1, tq] += ones^T @ pT (softmax row sums, same rhs stream)
    yT_norm = yT * bcast(1/r) (DVE copy -> GpSimd bcast -> DVE approx-recip)
    pairwise AllGather of this head's yT (overlaps later heads' compute)
  out    [t, c_half]: proj in two column halves; first half kk-major
    (tolerates late peer AllGather), second half m-major (drains output
    DMAs progressively so the kernel has no un-overlapped tail).
    Own heads' yT feed the proj straight from local DRAM (no AllGather
    dependency); wp rows are pre-swapped host-side into [own|peer] order
    so the SPMD program is parity-free, and peer tiles are fetched with
    parity-conditional DMAs.
Host side: slice/transpose/cast inputs per core, concat outputs.
"""

import numpy as np
import ml_dtypes

import concourse.bass as bass
import concourse.mybir as mybir
import concourse.tile as tile
from concourse import bacc
from concourse.bass_utils import run_bass_kernel_spmd
from concourse.dve_ops import RECIPROCAL_APPROX_NR

B, T, D = 4, 1024, 2048
H, DH = 16, 128
N_CORES = 8
TP = 2                      # head-halves per batch
HPC = H // TP               # heads per core = 8
CPC = HPC * DH              # channels per core = 1024
KC = D // 128               # contraction chunks = 16
XCH = 4                     # x/wv DMA chunks (4 k-slices each)
SCALE = 1.0 / float(np.sqrt(DH))

F32 = mybir.dt.float32
BF16 = mybir.dt.bfloat16

PAIRS = [[2 * i, 2 * i + 1] for i in range(B)]


def build_kernel():
    nc = bacc.Bacc("TRN2", target_bir_lowering=False, debug=False,
                   num_devices=N_CORES)

    xT_ap = nc.dram_tensor("xT", [D, T], BF16, kind="ExternalInput").ap()
    wq_ap = nc.dram_tensor("wq", [D, CPC], BF16, kind="ExternalInput").ap()
    wk_ap = nc.dram_tensor("wk", [D, CPC], BF16, kind="ExternalInput").ap()
    wv_ap = nc.dram_tensor("wv", [D, CPC], BF16, kind="ExternalInput").ap()
    wp_ap = nc.dram_tensor("wp", [D, CPC], BF16, kind="ExternalInput").ap()
    maskT_ap = nc.dram_tensor("maskT", [128, 128], BF16,
                              kind="ExternalInput").ap()
    out_ap = nc.dram_tensor("out", [T, CPC], F32, kind="ExternalOutput").ap()

    with tile.TileContext(nc) as tc:
        _body(nc, tc, xT_ap, wq_ap, wk_ap, wv_ap, wp_ap, maskT_ap, out_ap)
    nc.compile()
    return nc


def _body(nc, tc, xT_ap, wq_ap, wk_ap, wv_ap, wp_ap, maskT_ap, out_ap):
    Exp = mybir.ActivationFunctionType.Exp
    mult = mybir.AluOpType.mult

    with tc.tile_pool(name="const", bufs=1) as const, \
         tc.tile_pool(name="dram", bufs=HPC, space="DRAM") as dram, \
         tc.tile_pool(name="xa", bufs=8) as xa, \
         tc.tile_pool(name="wvs", bufs=4) as wvsp, \
         tc.tile_pool(name="wqk", bufs=4) as wqkp, \
         tc.tile_pool(name="wp", bufs=1) as wpp, \
         tc.tile_pool(name="vv", bufs=8) as vvp, \
         tc.tile_pool(name="qk", bufs=4) as qkp, \
         tc.tile_pool(name="pt", bufs=10) as ptp, \
         tc.tile_pool(name="yt", bufs=2) as ytp, \
         tc.tile_pool(name="nrm", bufs=1) as nrm, \
         tc.tile_pool(name="yf", bufs=2 * HPC) as yfp, \
         tc.tile_pool(name="osb", bufs=2) as osb, \
         tc.tile_pool(name="ps", bufs=8, space="PSUM") as pp:

        _psn = [0]

        def psum():
            _psn[0] += 1
            return pp.tile([128, 512], F32, tag="ps", name=f"ps{_psn[0]}")

        maskT = const.tile([128, 128], BF16, tag="maskT", name="maskT")
        ones_f32 = const.tile([128, 1], F32, tag="ones_f32", name="ones_f32")
        nc.vector.memset(ones_f32, 1.0)
        ones_col = const.tile([128, 1], BF16, tag="ones_col", name="ones_col")
        nc.scalar.copy(out=ones_col, in_=ones_f32)

        # ---- input DMAs ----
        # Per-queue DMA bandwidth is the startup bottleneck: spread the
        # x / wv chunks round-robin over the three DMA-capable queues
        # (sync/scalar/gpsimd) ordered by first-use time.
        xts = [xa.tile([128, 2, T], BF16, tag="xT", name=f"xt{c}")
               for c in range(8)]

        def load_x(eng, c):
            eng.dma_start(
                out=xts[c],
                in_=xT_ap[256 * c:256 * (c + 1), :].rearrange(
                    "(k p) t -> p k t", p=128))

        def xk(k):
            return xts[k // 2][:, k % 2, :]


        def load_wqk(hp):     # loads q+k weights for heads 2*hp, 2*hp+1
            tiles = []
            for w_ap, nm in ((wq_ap, "q"), (wk_ap, "k")):
                wt = wqkp.tile([128, KC, 256], BF16, tag="wqk",
                               name=f"w{nm}p{hp}")
                nc.gpsimd.dma_start(
                    out=wt,
                    in_=w_ap[:, 256 * hp:256 * (hp + 1)].rearrange(
                        "(k p) j -> p k j", p=128))
                tiles.append(wt)
            return tiles

        yt_loc = [dram.tile([128, T], BF16, tag="ytl", name=f"ytl{h}")
                  for h in range(HPC - 2)]
        yt_all = [dram.tile([TP, 128, T], BF16, tag="yta", name=f"yta{h}")
                  for h in range(HPC - 2)]
        yt67_loc = dram.tile([2, 128, T], BF16, tag="ytl67", name="ytl67")
        yt67_all = dram.tile([TP, 2, 128, T], BF16, tag="yta67",
                             name="yta67")

        vv = [vvp.tile([128, CPC], BF16, tag="vv", name=f"vv{j}")
              for j in range(8)]

        # ---- v natural [t, c]: stationary xT slices, moving wv ----
        # wv streamed in half-column chunks, re-fetched per ch pass
        wvts = [wvsp.tile([128, KC // XCH, 512], BF16, tag="wv",
                          name=f"wv{ch}_{c}")
                for ch in range(2) for c in range(XCH)]

        def load_wv(eng, i):
            ch, c = i // XCH, i % XCH
            eng.dma_start(
                out=wvts[i],
                in_=wv_ap[512 * c:512 * (c + 1),
                          512 * ch:512 * (ch + 1)].rearrange(
                    "(k p) j -> p k j", p=128))

        # interleaved by need time across the two HARDWARE DGE queues
        # (sync/SP and scalar/Act; gpsimd DMA is a slow software path):
        # x chunk c feeds k=2c..2c+1 (~3.4us per chunk), wv chunk i feeds
        # 4 k-iterations of one ch pass
        load_x(nc.sync, 0)
        load_wv(nc.scalar, 0)
        load_x(nc.sync, 1)
        load_x(nc.scalar, 2)
        load_wv(nc.sync, 1)
        load_x(nc.scalar, 3)
        load_x(nc.sync, 4)
        load_wv(nc.scalar, 2)
        load_x(nc.sync, 5)
        load_x(nc.scalar, 6)
        load_wv(nc.sync, 3)
        load_x(nc.scalar, 7)
        load_wv(nc.sync, 4)
        load_wv(nc.scalar, 5)
        load_wv(nc.sync, 6)
        load_wv(nc.scalar, 7)
        nc.gpsimd.dma_start(out=maskT, in_=maskT_ap)

        for ch in range(2):
            ps = [psum() for _ in range(8)]
            for k in range(KC):
                rhs = wvts[4 * ch + k // 4][:, k % 4, :]
                for tch in range(8):
                    nc.tensor.matmul(
                        ps[tch], xk(k)[:, 128 * tch:128 * (tch + 1)], rhs,
                        start=(k == 0), stop=(k == KC - 1))
            for tch in range(8):
                nc.scalar.copy(
                    out=vv[tch][:, 512 * ch:512 * (ch + 1)], in_=ps[tch])

        # ---- per-head: qk gemm + attention + per-head AllGather ----
        pid = nc.sync.partition_id()
        is_odd = pid % 2
        is_even = 1 - is_odd
        yfull = [None] * (2 * HPC)      # [h] = own head h, [HPC+h] = peer
        wts_next = load_wqk(0)
        # wp is not needed until the projection: issue it on the scalar
        # queue, which only reaches this instruction after the v-proj
        # copies -> it cannot steal startup DMA bandwidth
        wpt = wpp.tile([128, KC, CPC], BF16, tag="wp", name="wp")
        nc.scalar.dma_start(
            out=wpt, in_=wp_ap.rearrange("(k p) j -> p k j", p=128))
        for h in range(HPC):
            if h % 2 == 0:
                wts_cur = wts_next
                if h + 2 < HPC:
                    wts_next = load_wqk(h // 2 + 1)
            ho = 128 * (h % 2)
            qkT = []
            for wt, nm in ((wts_cur[0], "q"), (wts_cur[1], "k")):
                outT = qkp.tile([128, T], BF16, tag="qkT", name=f"{nm}T{h}")
                qkT.append(outT)
                for th in range(2):
                    ps = psum()
                    for k in range(KC):
                        nc.tensor.matmul(
                            ps, wt[:, k, ho:ho + 128],
                            xk(k)[:, 512 * th:512 * (th + 1)],
                            start=(k == 0), stop=(k == KC - 1))
                    nc.scalar.copy(
                        out=outT[:, 512 * th:512 * (th + 1)], in_=ps)
            qTh, kTh = qkT

            pts = []
            for j in range(8):
                pt = ptp.tile([128, T], BF16, tag="pT", name=f"pT{h}_{j}")
                pts.append(pt)
                if j < 4:
                    pieces = [(128 * j, 512 - 128 * j), (512, 512)]
                else:
                    pieces = [(128 * j, 1024 - 128 * j)]
                for off, cw in pieces:
                    sp = psum()
                    nc.tensor.matmul(
                        sp[:, :cw], kTh[:, 128 * j:128 * (j + 1)],
                        qTh[:, off:off + cw], start=True, stop=True)
                    nc.scalar.activation(
                        out=pt[:, off - 128 * j:off - 128 * j + cw],
                        in_=sp[:, :cw], func=Exp, scale=SCALE)
                # causal mask on the diagonal block (local cols 0:128)
                nc.vector.tensor_tensor(
                    out=pt[:, 0:128], in0=pt[:, 0:128], in1=maskT, op=mult)

            yt = ytp.tile([128, T], BF16, tag="yT", name=f"yt{h}")
            for g in range(2):
                tq0 = 512 * g
                jmax = 4 * (g + 1)
                yp = psum()
                rp = psum()
                for j in range(jmax):
                    lo = max(tq0, 128 * j)          # first valid tq
                    w = tq0 + 512 - lo
                    rhs = pts[j][:, lo - 128 * j:lo - 128 * j + w]
                    vblk = vv[j][:, 128 * h:128 * (h + 1)]
                    nc.tensor.matmul(
                        yp[:, lo - tq0:lo - tq0 + w], vblk, rhs,
                        start=(j == 0), stop=(j == jmax - 1))
                    nc.tensor.matmul(
                        rp[0:1, lo - tq0:lo - tq0 + w], ones_col, rhs,
                        start=(j == 0), stop=(j == jmax - 1))
                # softmax denom: psum -> sbuf -> bcast -> recip -> mult
                r_sb = nrm.tile([1, 512], F32, tag="r_sb", name=f"r_sb{h}_{g}")
                nc.vector.tensor_copy(out=r_sb, in_=rp[0:1, :])
                r_bc = nrm.tile([128, 512], F32, tag="r_bc", name=f"r_bc{h}_{g}")
                nc.gpsimd.partition_broadcast(r_bc, r_sb)
                rec = nrm.tile([128, 512], F32, tag="rec", name=f"rec{h}_{g}")
                nc.vector.reciprocal_approx_fast(out=rec, in_=r_bc)
                nc.vector._custom_dve(
                    RECIPROCAL_APPROX_NR, out=rec, in0=r_bc, in1=rec, s0=2.0)
                nc.vector.tensor_tensor(out=yt[:, tq0:tq0 + 512],
                                        in0=yp, in1=rec, op=mult)
            # ship this head's yT to the pair as soon as it's done; own
            # copy feeds the proj from local DRAM without any AG wait
            def fetch_peer(h2, src_aps):
                t2 = yfp.tile([128, T], BF16, tag="yfull",
                              name=f"yfpeer{h2}")
                nc.sync.dma_start(out=t2, in_=src_aps[1], cond=is_even)
                nc.sync.dma_start(out=t2, in_=src_aps[0], cond=is_odd)
                yfull[HPC + h2] = t2

            def fetch_own(h2, src_ap):
                t2 = yfp.tile([128, T], BF16, tag="yfull",
                              name=f"yfown{h2}")
                nc.sync.dma_start(out=t2, in_=src_ap)
                yfull[h2] = t2

            if h < HPC - 2:
                nc.sync.dma_start(out=yt_loc[h], in_=yt)
                fetch_own(h, yt_loc[h])
                nc.gpsimd.collective_compute(
                    "AllGather", mybir.AluOpType.bypass,
                    replica_groups=PAIRS,
                    ins=[yt_loc[h].opt()], outs=[yt_all[h].opt()])
                fetch_peer(h, [yt_all[h][0], yt_all[h][1]])
            else:
                nc.sync.dma_start(out=yt67_loc[h - (HPC - 2)], in_=yt)
                fetch_own(h, yt67_loc[h - (HPC - 2)])
                if h == HPC - 1:
                    nc.gpsimd.collective_compute(
                        "AllGather", mybir.AluOpType.bypass,
                        replica_groups=PAIRS,
                        ins=[yt67_loc.opt()], outs=[yt67_all.opt()])
                    for hh in range(2):
                        fetch_peer(HPC - 2 + hh,
                                   [yt67_all[0][hh], yt67_all[1][hh]])

        # ---- output projection out[t, c_half] = yT_full @ wp cols ----
        # consume own heads first (available without AG), peers after, in
        # head order (matches AG completion); wp rows are [own|peer].
        # first column half: kk-major (tolerates the last AllGather landing
        # mid-phase; psum holds all 8 t-chunks)
        ps0 = [psum() for _ in range(8)]
        for idx in range(2 * HPC):
            rhs = wpt[:, idx, 0:512]
            for m in range(8):
                nc.tensor.matmul(
                    ps0[m], yfull[idx][:, 128 * m:128 * (m + 1)], rhs,
                    start=(idx == 0), stop=(idx == 2 * HPC - 1))
        for m in range(8):
            ot = osb.tile([128, 512], F32, tag="ot", name=f"ot0_{m}")
            nc.scalar.copy(out=ot, in_=ps0[m])
            nc.sync.dma_start(
                out=out_ap[128 * m:128 * (m + 1), 0:512], in_=ot)
        # second column half: m-major (each t-chunk drains + DMAs as soon
        # as its 16 accumulations finish -> no output tail)
        for m in range(8):
            ps = psum()
            for idx in range(2 * HPC):
                nc.tensor.matmul(
                    ps, yfull[idx][:, 128 * m:128 * (m + 1)],
                    wpt[:, idx, 512:1024],
                    start=(idx == 0), stop=(idx == 2 * HPC - 1))
            ot = osb.tile([128, 512], F32, tag="ot", name=f"ot1_{m}")
            nc.scalar.copy(out=ot, in_=ps)
            nc.sync.dma_start(
                out=out_ap[128 * m:128 * (m + 1), 512:1024], in_=ot)


_NC_CACHE = None


def _get_nc():
    global _NC_CACHE
    if _NC_CACHE is None:
        _NC_CACHE = build_kernel()
    return _NC_CACHE


def kernel(x, w_qkv, w_proj, _trace=False, _trace_kwargs=None):
    bf16 = ml_dtypes.bfloat16
    x = np.asarray(x, dtype=np.float32)
    w_qkv = np.asarray(w_qkv, dtype=np.float32)
    w_proj = np.asarray(w_proj, dtype=np.float32)

    maskT = np.triu(np.ones((128, 128), dtype=np.float32)).astype(bf16)

    in_maps = []
    for c in range(N_CORES):
        b, hh = c // TP, c % TP
        cols = slice(hh * CPC, (hh + 1) * CPC)
        in_maps.append({
            "xT": np.ascontiguousarray(x[b].T).astype(bf16),
            "wq": np.ascontiguousarray(w_qkv[:, :D][:, cols]).astype(bf16),
            "wk": np.ascontiguousarray(
                w_qkv[:, D:2 * D][:, cols]).astype(bf16),
            "wv": np.ascontiguousarray(w_qkv[:, 2 * D:][:, cols]).astype(bf16),
            "wp": np.ascontiguousarray(np.concatenate(
                [w_proj[hh * CPC:(hh + 1) * CPC, cols],
                 w_proj[(1 - hh) * CPC:(2 - hh) * CPC, cols]],
                axis=0)).astype(bf16),
            "maskT": maskT,
        })

    nc = _get_nc()
    res = run_bass_kernel_spmd(nc, in_maps, list(range(N_CORES)),
                               trace=_trace, **(_trace_kwargs or {}))

    out = np.empty((B, T, D), dtype=np.float32)
    for c in range(N_CORES):
        b, hh = c // TP, c % TP
        out[b, :, hh * CPC:(hh + 1) * CPC] = res.results[c]["out"]
    if _trace:
        return out, res
    return out
